# revision 6
# baseline (speedup 1.0000x reference)
"""MultiHeadAttention Trainium2 kernel v4 (8-core SPMD, bf16 datapath).

v3 -> v4:
  - Weights host-packed to [128, N] tiles (4 KB contiguous per partition)
    -> one full-rate DMA per weight tensor instead of 8 descriptor-floor
    -limited ones; biases packed into a single [128, 10] tile.
  - out-proj and next-QT each get a dedicated 1-bank psum pool (no ring
    WAR against the S pipeline), and are interleaved half-chunk at a time
    (one matmul per attention iteration) so the PE tracks Act's exp pace.
Everything else as v3: Act-bound schedule, attention from ~36 us on.
"""

import numpy as np

B, T, C, H, D = 2, 2048, 1024, 16, 64
NCORES = 8
GROUPS = 4
HG = H // GROUPS        # heads per core (4)
DS = HG * D             # per-core projection slice width (256)
TCH = 512               # token chunk (psum bank = 512 fp32)
NTCH = T // TCH         # 4
NCC = C // 128          # 8 contraction chunks
NKT = T // 128          # 16 key tiles
SCALE = float(D) ** -0.5

_NC_CACHE = None
import os
PROBE = int(os.environ.get('KPROBE', '0'))


def _emit(ctx, tc, io):
    from concourse import mybir

    nc = tc.nc
    f32 = mybir.dt.float32
    bf16 = mybir.dt.bfloat16
    EXP = mybir.ActivationFunctionType.Exp

    persist = ctx.enter_context(tc.tile_pool(name="persist", bufs=1))

    def ptile(tag, shape, dt=f32):
        return persist.tile(shape, dt, tag=tag, name=tag)

    # --- DMAs in dependency order (all on SP; Act stays exp-only) -------
    # weights are host-packed: wX[p, cc*DS + d] = W.T[cc*128+p, d]
    wk = ptile("wk", [128, NCC * DS], bf16)
    nc.sync.dma_start(wk[:], io["wk"][:, :])
    xk = []
    for cc in range(NCC):
        t_ = ptile(f"xkt{cc}", [128, T], bf16)
        eng = nc.sync if cc % 2 == 0 else nc.gpsimd
        eng.dma_start(t_[:], io["xkt"][cc * 128:(cc + 1) * 128, :])
        xk.append(t_)
    wv = ptile("wv", [128, NCC * DS], bf16)
    nc.sync.dma_start(wv[:], io["wv"][:, :])
    xv = []
    for cc in range(NCC):
        t_ = ptile(f"xvt{cc}", [128, T], bf16)
        eng = nc.sync if cc % 2 == 0 else nc.gpsimd
        eng.dma_start(t_[:], io["xvt"][cc * 128:(cc + 1) * 128, :])
        xv.append(t_)
    wq = ptile("wq", [128, NCC * DS], bf16)
    nc.sync.dma_start(wq[:], io["wq"][:, :])
    # xqt streamed per (tc, cc) so q-chunk 0 lands early
    xq = [ptile(f"xqt{cc}", [128, T], bf16) for cc in range(NCC)]
    for cc in range(NCC):
        eng = nc.sync if cc % 2 == 0 else nc.gpsimd
        eng.dma_start(xq[cc][:, 0:TCH], io["xqt"][cc * 128:(cc + 1) * 128,
                                                  0:TCH])
    # wot[p, dc*C + co] = Wo.T[dc*128+p, co]
    wot = ptile("wot", [128, 2 * C], bf16)
    nc.sync.dma_start(wot[:], io["wot"][:, :])
    # bias[p, 0:2] = bq halves, bias[p, 2:10] = bo_eff chunks
    biasp = ptile("biasp", [128, 10])
    nc.sync.dma_start(biasp[:], io["biasp"][:, :])
    for tci in range(1, NTCH):
        for cc in range(NCC):
            eng = nc.sync if cc % 2 == 0 else nc.gpsimd
            eng.dma_start(
                xq[cc][:, tci * TCH:(tci + 1) * TCH],
                io["xqt"][cc * 128:(cc + 1) * 128,
                          tci * TCH:(tci + 1) * TCH])

    def wsl(w, cc, co):
        return w[:, cc * DS + co * 128:cc * DS + (co + 1) * 128]

    QT = [ptile(f"qt{i}", [128, T], bf16) for i in range(2)]
    KT = [ptile(f"kt{i}", [128, T], bf16) for i in range(2)]
    VN = [ptile(f"vn{i}", [128, HG * 128], bf16) for i in range(NKT)]
    # warm the exp table off the critical path
    warm = ptile("warm", [1, 2])
    nc.vector.memset(warm[:], 0.0)
    nc.scalar.activation(warm[:, 1:2], warm[:, 0:1], EXP)
    for kt in range(NKT):
        dst = VN[kt][:].rearrange("p (h c) -> p h c", h=HG)[:, :, 64:128]
        nc.vector.memset(dst, 1.0)

    # --- stage A': KT, VN, QT[tc0] --------------------------------------
    aps_cm = tc.tile_pool(name="aps", bufs=8, space="PSUM")
    aps = aps_cm.__enter__()

    # KT: co-outer, 4 parallel psums consume xkt tiles as they arrive
    for co in range(2):
        kps = [aps.tile([128, TCH], f32, tag="aps", name="kps")
               for _ in range(NTCH)]
        for cc in range(NCC):
            for tci in range(NTCH):
                nc.tensor.matmul(
                    kps[tci][:],
                    lhsT=wsl(wk, cc, co),
                    rhs=xk[cc][:, tci * TCH:(tci + 1) * TCH],
                    start=(cc == 0), stop=(cc == NCC - 1))
        for tci in range(NTCH):
            nc.vector.tensor_copy(
                KT[co][:, tci * TCH:(tci + 1) * TCH], kps[tci][:])

    # VN: 2 waves of 4 kt-pair psums, cc-outer (consumes xvt as it lands);
    # QT tc0 squeezed between the waves (right when xqt tc0 lands) so
    # attention can start while VN wave 1 computes
    def vn_wave(wave):
        vps = [aps.tile([128, TCH], f32, tag="aps", name="vps")
               for _ in range(4)]
        for u in range(2):  # one accumulation group per bank at a time
            for cc in range(NCC):
                for p_ in range(4):
                    kt0 = wave * 8 + 2 * p_
                    nc.tensor.matmul(
                        vps[p_][:, u * DS:(u + 1) * DS],
                        lhsT=xv[cc][:, (kt0 + u) * 128:(kt0 + u + 1) * 128],
                        rhs=wv[:, cc * DS:(cc + 1) * DS],
                        start=(cc == 0), stop=(cc == NCC - 1))
        for p_ in range(4):
            kt0 = wave * 8 + 2 * p_
            for u in range(2):
                src3 = vps[p_][:, u * DS:(u + 1) * DS].rearrange(
                    "p (h d) -> p h d", h=HG)
                dst3 = VN[kt0 + u][:].rearrange(
                    "p (h c) -> p h c", h=HG)[:, :, 0:64]
                nc.vector.tensor_copy(dst3, src3)

    vn_wave(0)
    vn_wave(1)
    qps = [aps.tile([128, TCH], f32, tag="aps", name="qtps")
           for _ in range(2)]
    for cc in range(NCC):
        for co in range(2):
            nc.tensor.matmul(
                qps[co][:], lhsT=wsl(wq, cc, co), rhs=xq[cc][:, 0:TCH],
                start=(cc == 0), stop=(cc == NCC - 1))
    for co in range(2):
        nc.vector.tensor_scalar_add(
            QT[co][:, 0:TCH], qps[co][:], biasp[:, co:co + 1])
    aps_cm.__exit__(None, None, None)

    # --- stage B: attention with interleaved out-proj + next-QT ---------
    with tc.tile_pool(name="sps", bufs=2, space="PSUM") as sps, \
         tc.tile_pool(name="otps", bufs=2, space="PSUM") as otps, \
         tc.tile_pool(name="qtp", bufs=1, space="PSUM") as qtp, \
         tc.tile_pool(name="ppp", bufs=1, space="PSUM") as ppp, \
         tc.tile_pool(name="expsb", bufs=6) as expsb, \
         tc.tile_pool(name="otsb", bufs=4) as otsbp, \
         tc.tile_pool(name="recsb", bufs=4) as recp, \
         tc.tile_pool(name="outsb", bufs=3) as outsbp:

        osb_prev = None

        def outproj_steps(qc_prev, osb_pair, pool=None, tag="pp"):
            """One out-proj ct chunk split into 2 one-matmul steps."""
            pool = ppp if pool is None else pool
            qcols = slice(qc_prev * TCH, (qc_prev + 1) * TCH)
            state = {}

            def step_dc(ct, dc):
                if dc == 0:
                    state[ct] = pool.tile([128, TCH], f32, tag=tag,
                                          name="prj")
                pp = state[ct]
                nc.tensor.matmul(
                    pp[:],
                    lhsT=wot[:, dc * C + ct * 128:dc * C + (ct + 1) * 128],
                    rhs=osb_pair[dc][:],
                    start=(dc == 0), stop=(dc == 1))
                if dc == 1:
                    ob = outsbp.tile([128, TCH], bf16, tag="ob", name="ob")
                    nc.vector.tensor_scalar_add(
                        ob[:], pp[:], biasp[:, 2 + ct:3 + ct])
                    nc.sync.dma_start(
                        io["out_t"][ct * 128:(ct + 1) * 128, qcols], ob[:])
                    del state[ct]

            return [(step_dc, ct, dc) for ct in range(NCC) for dc in (0, 1)]

        def qt_steps(tci):
            state = {}

            def step(co, cc):
                if cc == 0:
                    state[co] = qtp.tile([128, TCH], f32, tag="qt",
                                         name="qtps")
                ps = state[co]
                nc.tensor.matmul(
                    ps[:], lhsT=wsl(wq, cc, co),
                    rhs=xq[cc][:, tci * TCH:(tci + 1) * TCH],
                    start=(cc == 0), stop=(cc == NCC - 1))
                if cc == NCC - 1:
                    nc.vector.tensor_scalar_add(
                        QT[co][:, tci * TCH:(tci + 1) * TCH],
                        ps[:], biasp[:, co:co + 1])
                    del state[co]

            return [(step, co, cc) for co in range(2) for cc in range(NCC)]

        for qc in range(NTCH):
            qcols = slice(qc * TCH, (qc + 1) * TCH)
            # filler schedule: iters 0..15 -> out-proj(qc-1) half-chunks;
            # iters 16..31 -> QT(qc+1) single matmuls
            fillers = []
            if osb_prev is not None:
                fillers.extend([None] * 4)
                fillers.extend(outproj_steps(qc - 1, osb_prev))
            else:
                fillers.extend([None] * 16)
            if qc + 1 < NTCH:
                fillers.extend(qt_steps(qc + 1))
            # 64 fill slots per qc (2 per iteration)
            fillers.extend([None] * (64 - len(fillers)))

            # software-pipelined attention: the AV pair for iteration i is
            # emitted two iterations later (cross-engine semaphores fully
            # settled before the PE reaches the dependent matmul), with a
            # filler between the same-bank AV pair
            otp = {}          # (pr, hh) -> psum accumulator
            osb = {}          # pr -> normalized sbuf tile
            pending = []      # [(pr, g, hh, es), ...] oldest first

            def fill():
                if fillers:
                    f = fillers.pop(0)
                    if f is not None:
                        f[0](*f[1:])

            def flush_one():
                if not pending:
                    return
                pr, g, hh, es = pending.pop(0)
                h = pr * 2 + hh
                if g == 0:
                    otp[(pr, hh)] = otps.tile([128, TCH], f32, tag="ot",
                                              name="ot")
                acc = otp[(pr, hh)]
                for j in range(2):
                    kt = 2 * g + j
                    nc.tensor.matmul(
                        acc[:, :],
                        lhsT=VN[kt][:, h * 128:(h + 1) * 128],
                        rhs=es[:, j * TCH:(j + 1) * TCH],
                        start=(g == 0 and j == 0),
                        stop=(g == NKT // 2 - 1 and j == 1))
                    if j == 0:
                        fill()
                if g == NKT // 2 - 1 and hh == 1:
                    # normalize: psum rows 64-127 hold the denominator
                    ob = otsbp.tile([128, TCH], bf16, tag="otsb",
                                    name="otsb")
                    for h2 in range(2):
                        a2 = otp.pop((pr, h2))
                        rec = recp.tile([64, TCH], f32, tag="rec",
                                        name="rec")
                        nc.vector.reciprocal(rec[:], a2[64:128, :])
                        nc.vector.tensor_mul(
                            ob[h2 * 64:(h2 + 1) * 64, :],
                            a2[0:64, :], rec[:])
                    osb[pr] = ob

            for pr in range(2):
                for g in range(NKT // 2):
                    for hh in range(2):
                        rows = slice(hh * 64, (hh + 1) * 64)
                        S = sps.tile([128, 2 * TCH], f32, tag="s", name="s")
                        for j in range(2):
                            kt = 2 * g + j
                            nc.tensor.matmul(
                                S[:, j * TCH:(j + 1) * TCH],
                                lhsT=KT[pr][rows, kt * 128:(kt + 1) * 128],
                                rhs=QT[pr][rows, qcols],
                                start=True, stop=True)
                        es = expsb.tile([128, 2 * TCH], bf16, tag="es",
                                        name="es")
                        nc.scalar.activation(es[:], S[:], EXP, scale=SCALE)
                        if len(pending) >= 3:
                            flush_one()
                        fill()
                        pending.append((pr, g, hh, es))
            while pending:
                flush_one()
            for f in fillers:
                if f is not None:
                    f[0](*f[1:])
            osb_prev = [osb[0], osb[1]]

        # tail: deeper psum ring (otps is free after the last norm)
        for f in outproj_steps(NTCH - 1, osb_prev, pool=otps, tag="ot"):
            f[0](*f[1:])


def build_nc(reps=1):
    from contextlib import ExitStack

    import concourse.tile as tile
    from concourse import bacc, mybir

    f32 = mybir.dt.float32
    bf16 = mybir.dt.bfloat16
    nc = bacc.Bacc("TRN2", target_bir_lowering=False, debug=False,
                   num_devices=NCORES)
    io = {}
    for name in ("xqt", "xkt", "xvt"):
        io[name] = nc.dram_tensor(name, [C, T], bf16,
                                  kind="ExternalInput").ap()
    for name in ("wq", "wk", "wv"):
        io[name] = nc.dram_tensor(name, [128, NCC * DS], bf16,
                                  kind="ExternalInput").ap()
    io["wot"] = nc.dram_tensor("wot", [128, 2 * C], bf16,
                               kind="ExternalInput").ap()
    io["biasp"] = nc.dram_tensor("biasp", [128, 10], f32,
                                 kind="ExternalInput").ap()
    io["out_t"] = nc.dram_tensor("out_t", [C, T], bf16,
                                 kind="ExternalOutput").ap()

    with tile.TileContext(nc) as tc:
        if reps == 1:
            with ExitStack() as ctx:
                _emit(ctx, tc, io)
        else:
            with tc.For_i(0, reps, 1):
                with ExitStack() as ctx:
                    _emit(ctx, tc, io)
    nc.compile()
    return nc


def get_nc():
    global _NC_CACHE
    if _NC_CACHE is None:
        _NC_CACHE = build_nc()
    return _NC_CACHE


def _pack_w(wt):
    """[NCC*128, DS] -> [128, NCC*DS] with 4KB-contiguous partition rows."""
    ncc = wt.shape[0] // 128
    return np.ascontiguousarray(
        wt.reshape(ncc, 128, wt.shape[1]).transpose(1, 0, 2).reshape(
            128, ncc * wt.shape[1]))


def make_in_maps(q, k, v, Wq, bq, Wk, bk, Wv, bv, Wo, bo):
    import ml_dtypes

    bfd = ml_dtypes.bfloat16
    q, k, v = (np.asarray(x, np.float32) for x in (q, k, v))
    Wq, Wk, Wv, Wo = (np.asarray(x, np.float32) for x in (Wq, Wk, Wv, Wo))
    bq, bk, bv, bo = (np.asarray(x, np.float32) for x in (bq, bk, bv, bo))
    # bk is softmax-invariant; bv passes through attention into the
    # out-projection bias
    bo_eff = bo + Wo @ bv
    xt = {}
    for b in range(B):
        xt[("q", b)] = np.ascontiguousarray(q[b].T).astype(bfd)
        xt[("k", b)] = np.ascontiguousarray(k[b].T).astype(bfd)
        xt[("v", b)] = np.ascontiguousarray(v[b].T).astype(bfd)
    in_maps = []
    for core in range(NCORES):
        b, g = divmod(core, GROUPS)
        sl = slice(g * DS, (g + 1) * DS)
        biasp = np.zeros((128, 10), np.float32)
        biasp[:, 0:2] = bq[sl].reshape(2, 128).T
        if g == 0:
            biasp[:, 2:10] = bo_eff.reshape(8, 128).T
        in_maps.append({
            "xqt": xt[("q", b)],
            "xkt": xt[("k", b)],
            "xvt": xt[("v", b)],
            "wq": _pack_w(Wq[sl, :].T).astype(bfd),
            "wk": _pack_w(Wk[sl, :].T).astype(bfd),
            "wv": _pack_w(Wv[sl, :].T).astype(bfd),
            "wot": _pack_w(np.ascontiguousarray(Wo[:, sl].T)).astype(bfd),
            "biasp": biasp,
        })
    return in_maps


def combine(results):
    out = np.zeros((B, T, C), np.float32)
    for core in range(NCORES):
        b, _ = divmod(core, GROUPS)
        out[b] += results[core]["out_t"].T.astype(np.float32)
    return out


def kernel(q, k, v, Wq, bq, Wk, bk, Wv, bv, Wo, bo):
    from concourse.bass_utils import run_bass_kernel_spmd

    nc = get_nc()
    in_maps = make_in_maps(q, k, v, Wq, bq, Wk, bk, Wv, bv, Wo, bo)
    res = run_bass_kernel_spmd(nc, in_maps, core_ids=list(range(NCORES)))
    return combine(res.results)
